# revision 22
# baseline (speedup 1.0000x reference)
"""Trainium2 Bass kernel for the EntangledInterferenceLayer problem (v2).

Math transformations on host (numpy float64, exact up to fp rounding):
  * HxH entanglement mix commutes with RoPE -> folded into Q/K weights+biases.
  * Per-head phase shift cancels in q*conj(k) -> dropped.
  * 1/sqrt(head_dim) folded into Q weights/bias.
  * V-projection bias contributes bv @ Wo to every row -> folded into out bias.

Sharding (8 cores): core = (batch b, head-group g of 4 heads). Device compute
in bf16 (fp32 PSUM/softmax internals); rel tolerance is 2e-2.

v2 design vs the original baseline:
  * Projections computed per head-PAIR: stationary [128, h0|h1 x 64 dims],
    PSUM rows = [h0 rot|nr | h1 rot|nr]; bias added by ScalarE (Identity with
    per-partition bias AP); RoPE applied by DVE (stream_shuffle pairswap +
    cos/sin tables with 1/0 rows for non-rot dims) writing directly into the
    per-head Q/K1 stacks via partition-offset-shifted adds. No bias matmuls,
    no SBUF->SBUF staging DMAs.
  * K2 = [-ki | kr] built with two 64-row DVE copies per head.
  * V written straight from PSUM into the persistent V tile (strided ACT copy).
  * All weights/x pre-packed on host into the exact SBUF layouts (contiguous
    DMA), loaded once.
  * Attention trimmed to the causal support: kv-tile (q >= 128*off) column
    ranges only (~25% less score/softmax/AV work); sqrt/exp batched per
    head-pair so ACT reloads tables only 2x per pair.
  * Softmax denominator: ones-column in Vr; reciprocal via gpsimd
    partition_broadcast + reciprocal_approx_fast; normalized outputs written
    into the local slot of the recv buffer as pure-real/imag 2-head chunks.
  * Comms: remote_dma_broadcast (XOR-relative dests) pushes each pair's
    normalized output [128, 2, 512] bf16 directly into the 3 peers' recv SBUF
    slots; arrival via remote semaphore (+2/send); receive side waits inside
    a tile_critical so the Tile scheduler doesn't deadlock. A tiny AllReduce
    barrier + sem_clear at kernel start makes re-execution of the NEFF safe;
    sends are ordered after the barrier via data edges (barrier result DMA'd
    into each payload chunk corner, overwritten by the normalize muls).
  * Output projection contracts recv chunks (K=128 fully used since chunks
    are pure real/imag) with per-core slot-permuted Wo (host knows each
    core's XOR peer map), + ScalarE bias, f32 out.
"""

import math
import os

import numpy as np

_NO_SEND = os.environ.get("ANT_NO_SEND") == "1"
_NO_WAIT = os.environ.get("ANT_NO_WAIT") == "1"

B, S, DIM = 2, 1024, 1024
HEADS, HD, ROTD = 16, 64, 32
GH = 4  # heads per core
ODC = 256  # out-dim columns per core
NCORES = 8

_PAIRSWAP = [i ^ 1 for i in range(32)]


def _register_magsq():
    """Register a fused custom DVE op: out = (in0^2 + in1^2) * imm2."""
    import numpy as np
    from concourse import dve_ops as DO
    from concourse.dve_spec import Spec, Src0, Src1, C2, sq, lower

    if "ANT_MAGSQ" in DO._SUB_OPCODE_FOR_NAME:
        return next(o for o in DO.OPS if o.name == "ANT_MAGSQ")
    spec = Spec(
        body=(sq(Src0) + sq(Src1)) * C2,
        reference=lambda in0, in1, s0, s1, imm2: (
            in0.astype(np.float32) ** 2 + in1.astype(np.float32) ** 2
        )
        * np.float32(imm2),
    )
    opcode = DO._CUSTOM_DVE_ROW_BASE + len(DO.OPS)
    DO._SUB_OPCODE_FOR_NAME["ANT_MAGSQ"] = opcode
    shas = {}
    for ver in ("v3", "v4"):
        try:
            s = DO.DveOpSpec(
                name="ANT_MAGSQ", opcode=opcode, uops=lower(spec, ver=ver), rd1_en=True
            )
            shas[ver] = s.sha(ver)
        except Exception:
            pass
    op = DO.DveOp("ANT_MAGSQ", spec, subdim=False, uops_sha=shas)
    DO.OPS.append(op)
    DO.CUSTOM_DVE_SPECS["ANT_MAGSQ"] = spec
    return op


def _build(gt: float, groups=None):
    import concourse.mybir as mybir
    import concourse.tile as tile
    from concourse import bacc

    f32 = mybir.dt.float32
    bf16 = mybir.dt.bfloat16
    AF = mybir.ActivationFunctionType
    magsq = _register_magsq()

    nc = bacc.Bacc("TRN2", target_bir_lowering=False, num_devices=NCORES)
    if groups is None:
        groups = [[0, 1, 2, 3], [4, 5, 6, 7]]

    xr_d = nc.dram_tensor("xr", [2, 128, 8, 512], bf16, kind="ExternalInput")
    xi_d = nc.dram_tensor("xi", [2, 128, 8, 512], bf16, kind="ExternalInput")
    wqk_d = {
        nm: nc.dram_tensor(nm, [128, 2, 8, 128], bf16, kind="ExternalInput")
        for nm in ("wqr", "wqi", "wkr", "wki")
    }
    wv_d = {
        nm: nc.dram_tensor(nm, [128, 8, 256], bf16, kind="ExternalInput")
        for nm in ("wvr", "wvi")
    }
    wo_d = {
        nm: nc.dram_tensor(nm, [128, 4, 2, 256], bf16, kind="ExternalInput")
        for nm in ("wor", "woi")
    }
    bqk_d = nc.dram_tensor("bqk", [128, 2, 4], f32, kind="ExternalInput")
    bo_d = nc.dram_tensor("bo", [128, 2, 2], f32, kind="ExternalInput")
    cos_d = nc.dram_tensor("cosd", [128, 1024], bf16, kind="ExternalInput")
    sin_d = nc.dram_tensor("sind", [128, 1024], bf16, kind="ExternalInput")
    o_r = nc.dram_tensor("o_r", [ODC, S], f32, kind="ExternalOutput")
    o_i = nc.dram_tensor("o_i", [ODC, S], f32, kind="ExternalOutput")

    rsem = [nc.alloc_semaphore("rsem0"), nc.alloc_semaphore("rsem1")]
    lsem = [nc.alloc_semaphore("lsem0"), nc.alloc_semaphore("lsem1")]

    def mm(out, lhsT, rhs, start, stop):
        nc.tensor.matmul(out, lhsT=lhsT, rhs=rhs, start=start, stop=stop)

    with tile.TileContext(nc) as tc:
        with (
            tc.tile_pool(name="consts", bufs=1) as consts,
            tc.tile_pool(name="persist", bufs=1) as persist,
            tc.tile_pool(name="xp", bufs=4) as xp,
            tc.tile_pool(name="ptmp", bufs=3) as ptmp,
            tc.tile_pool(name="cp", bufs=4) as cp,
            tc.tile_pool(name="sqp", bufs=17) as sqp,
            tc.tile_pool(name="etp", bufs=17) as etp,
            tc.tile_pool(name="rp", bufs=2) as rp,
            tc.tile_pool(name="op", bufs=2) as op,
            tc.tile_pool(name="ps", bufs=8, space="PSUM") as ps,
            tc.tile_pool(name="dram", bufs=1, space="DRAM") as dram,
        ):
            # ---- persistent SBUF state ----
            Q = persist.tile([128, GH, S], bf16, tag="Q")
            K1 = persist.tile([128, GH, S], bf16, tag="K1")
            K2 = persist.tile([128, GH, S], bf16, tag="K2")
            Vr = persist.tile([128, 8, GH, 65], bf16, tag="Vr")
            Vi = persist.tile([128, 8, GH, 64], bf16, tag="Vi")
            recv = [
                persist.tile([128, 4, 4, 512], bf16, tag=f"recv{qc}",
                             name=f"recv{qc}")
                for qc in range(2)
            ]

            # ---- init: sem clear + cross-core barrier (re-exec safety) ----
            nc.gpsimd.sem_clear(rsem[0])
            nc.gpsimd.sem_clear(rsem[1])
            nc.gpsimd.sem_clear(lsem[0])
            nc.gpsimd.sem_clear(lsem[1])
            def emit_barrier():
                bar_in = dram.tile([1, 4], f32, tag="bar_in", name="bar_in")
                bar_out = dram.tile([1, 4], f32, tag="bar_out", name="bar_out")
                barsb = consts.tile([1, 4], f32, tag="barsb")
                nc.vector.memset(barsb, 1.0)
                nc.sync.dma_start(bar_in, barsb)
                nc.gpsimd.collective_compute(
                    "AllReduce",
                    mybir.AluOpType.add,
                    replica_groups=groups,
                    ins=[bar_in[:].opt()],
                    outs=[bar_out[:].opt()],
                )
                # data edges: gate every send payload chunk on the barrier
                for qc in range(2):
                    for ch in range(4):
                        nc.sync.dma_start(
                            recv[qc][0:1, 0, ch, 0:8].bitcast(f32), bar_out
                        )

            # ---- constants / weights (loaded once) ----
            nc.vector.memset(Vr[:, :, :, 64:65], 1.0)
            eps_t = consts.tile([128, 1], f32, tag="eps")
            nc.vector.memset(eps_t, 1e-6 * float(gt) * float(gt))
            cos_sb = consts.tile([128, 1024], bf16, tag="cos")
            nc.sync.dma_start(cos_sb, cos_d[:, :])
            sin_sb = consts.tile([128, 1024], bf16, tag="sin")
            nc.sync.dma_start(sin_sb, sin_d[:, :])
            bqk_sb = {}
            for pair in range(2):
                for kind in range(4):
                    t = consts.tile([128, 1], f32, tag=f"bqk{pair}{kind}",
                                    name=f"bqk{pair}{kind}")
                    nc.sync.dma_start(t, bqk_d[:, pair:pair + 1, kind])
                    bqk_sb[(pair, kind)] = t
            bo_sb = {}
            for ri in range(2):
                for odt in range(2):
                    t = consts.tile([128, 1], f32, tag=f"bo{ri}{odt}",
                                    name=f"bo{ri}{odt}")
                    nc.sync.dma_start(t, bo_d[:, ri:ri + 1, odt])
                    bo_sb[(ri, odt)] = t
            wqk_sb = {}
            wv_sb = {}
            wo_sb = {}

            def load_qk_weights():
                for nm in wqk_d:
                    t = consts.tile([128, 2, 8, 128], bf16, tag=nm, name=nm)
                    nc.sync.dma_start(t, wqk_d[nm][:, :, :, :])
                    wqk_sb[nm] = t

            def load_v_weights():
                for nm in wv_d:
                    t = consts.tile([128, 8, 256], bf16, tag=nm, name=nm)
                    nc.sync.dma_start(t, wv_d[nm][:, :, :])
                    wv_sb[nm] = t

            def load_o_weights():
                for nm in wo_d:
                    t = consts.tile([128, 4, 2, 256], bf16, tag=nm, name=nm)
                    nc.sync.dma_start(t, wo_d[nm][:, :, :, :])
                    wo_sb[nm] = t

            def proj_pair(pair, w_sb, bias_kind, x_tiles, dst, half, csl):
                """One [128,512] PSUM pair-tile -> rope -> dst rows."""
                pst = ps.tile([128, 512], f32, tag="ps")
                for kt in range(8):
                    mm(pst, w_sb[:, pair, kt, :], x_tiles[kt],
                       start=(kt == 0), stop=(kt == 7))
                tb = ptmp.tile([128, 512], bf16, tag="tb")
                nc.scalar.activation(
                    tb, pst, AF.Identity, bias=bqk_sb[(pair, bias_kind)]
                )
                sh = ptmp.tile([128, 512], bf16, tag="sh")
                nc.vector.stream_shuffle(sh, tb, mask=_PAIRSWAP)
                nc.vector.tensor_mul(sh, sh, sin_sb[:, csl])
                t2 = ptmp.tile([128, 512], bf16, tag="t2")
                nc.vector.tensor_mul(t2, tb, cos_sb[:, csl])
                h0, h1 = 2 * pair, 2 * pair + 1
                r0 = half * 64
                nc.vector.tensor_add(
                    dst[r0:r0 + 64, h0, csl], t2[0:64, :], sh[0:64, :]
                )
                nc.vector.tensor_add(
                    dst[r0:r0 + 64, h1, csl], t2[64:128, :], sh[64:128, :]
                )

            x_tiles = {}

            def load_x(c):
                xr_b = xp.tile([128, 8, 512], bf16, tag="xt", name=f"xr{c}")
                xi_b = xp.tile([128, 8, 512], bf16, tag="xt", name=f"xi{c}")
                # split loads so the first matmul only waits for its k-chunks
                for q4 in range(4):
                    nc.sync.dma_start(
                        xr_b[:, 2 * q4:2 * q4 + 2, :],
                        xr_d[c, :, 2 * q4:2 * q4 + 2, :],
                    )
                for q4 in range(4):
                    nc.sync.dma_start(
                        xi_b[:, 2 * q4:2 * q4 + 2, :],
                        xi_d[c, :, 2 * q4:2 * q4 + 2, :],
                    )
                x_tiles[c] = (xr_b, xi_b)

            def proj_c(c):
                csl = slice(c * 512, (c + 1) * 512)
                xr_b, xi_b = x_tiles[c]
                xr_t = [xr_b[:, kt, :] for kt in range(8)]
                xi_t = [xi_b[:, kt, :] for kt in range(8)]
                if c == 0:
                    load_qk_weights()
                for pair in range(2):
                    proj_pair(pair, wqk_sb["wqr"], 0, xr_t, Q, 0, csl)
                    proj_pair(pair, wqk_sb["wqi"], 1, xi_t, Q, 1, csl)
                    proj_pair(pair, wqk_sb["wkr"], 2, xr_t, K1, 0, csl)
                    proj_pair(pair, wqk_sb["wki"], 3, xi_t, K1, 1, csl)
                    for h in (2 * pair, 2 * pair + 1):
                        nc.vector.tensor_scalar_mul(
                            K2[0:64, h, csl], K1[64:128, h, csl], -1.0
                        )
                        nc.vector.tensor_copy(K2[64:128, h, csl], K1[0:64, h, csl])
                if c == 0:
                    load_v_weights()
                # V projections: out [tok, vdim] (x as stationary)
                for w_sb, Vt, x_t, w65 in (
                    (wv_sb["wvr"], Vr, xr_t, True),
                    (wv_sb["wvi"], Vi, xi_t, False),
                ):
                    for tl in range(4):
                        tt = c * 4 + tl
                        pv = ps.tile([128, 256], f32, tag="ps")
                        for kt in range(8):
                            mm(pv, x_t[kt][:, tl * 128:(tl + 1) * 128],
                               w_sb[:, kt, :], start=(kt == 0), stop=(kt == 7))
                        dst = Vt[:, tt, :, 0:64] if w65 else Vt[:, tt, :, :]
                        nc.scalar.activation(dst, pv, AF.Identity)

            def qrange(qc, kvt):
                off = kvt - qc * 4
                qlo = 128 * off if off > 0 else 0
                return off, qlo

            def attn_qc(qc):
                nkv = 4 * (qc + 1)
                qbase = qc * 512
                tiles = {0: [], 1: []}
                ets = {}

                def scores_pair(pair):
                    for h in (2 * pair, 2 * pair + 1):
                        for kvt in range(nkv):
                            off, qlo = qrange(qc, kvt)
                            qs = slice(qbase + qlo, qbase + 512)
                            ksl = slice(kvt * 128, (kvt + 1) * 128)
                            psr = ps.tile([128, 512], f32, tag="ps")
                            mm(psr[:, qlo:], K1[:, h, ksl], Q[:, h, qs],
                               start=True, stop=True)
                            psi = ps.tile([128, 512], f32, tag="ps")
                            mm(psi[:, qlo:], K2[:, h, ksl], Q[:, h, qs],
                               start=True, stop=True)
                            c1 = cp.tile([128, 512], bf16, tag="c1")
                            nc.vector.tensor_copy(c1[:, qlo:], psi[:, qlo:])
                            sq = sqp.tile([128, 512], bf16, tag="sq")
                            nc.vector._custom_dve(
                                magsq, out=sq[:, qlo:], in0=psr[:, qlo:],
                                in1=c1[:, qlo:], imm2=float(gt) * float(gt),
                            )
                            tiles[pair].append((h, kvt, qlo, sq))

                def actchain_pair(pair):
                    # batched ACT passes (one table load per function)
                    for h, kvt, qlo, sq in tiles[pair]:
                        nc.scalar.activation(
                            sq[:, qlo:], sq[:, qlo:], AF.Sqrt, bias=eps_t
                        )
                    for h, kvt, qlo, sq in tiles[pair]:
                        et = etp.tile([128, 512], bf16, tag="et")
                        nc.scalar.activation(et[:, qlo:], sq[:, qlo:], AF.Exp)
                        ets[(h, kvt)] = et

                def avnorm_pair(pair):
                    for h, kvt, qlo, sq in tiles[pair]:
                        if kvt - qc * 4 >= 0:
                            et = ets[(h, kvt)]
                            nc.gpsimd.affine_select(
                                out=et[:, qlo:],
                                in_=et[:, qlo:],
                                compare_op=mybir.AluOpType.is_ge,
                                fill=0.0,
                                base=0,
                                channel_multiplier=-1,
                                pattern=[[1, 512 - qlo]],
                            )
                    for sub, h in enumerate((2 * pair, 2 * pair + 1)):
                        avr = ps.tile([65, 512], f32, tag="ps")
                        avi = ps.tile([64, 512], f32, tag="ps")
                        for kvt in range(nkv):
                            off, qlo = qrange(qc, kvt)
                            et = ets[(h, kvt)]
                            mm(avr[:, qlo:], Vr[:, kvt, h, :], et[:, qlo:],
                               start=(kvt == 0), stop=(kvt == nkv - 1))
                            mm(avi[:, qlo:], Vi[:, kvt, h, :], et[:, qlo:],
                               start=(kvt == 0), stop=(kvt == nkv - 1))
                        den1 = rp.tile([1, 512], f32, tag="den1")
                        nc.vector.tensor_copy(den1, avr[64:65, :])
                        denb = rp.tile([64, 512], f32, tag="denb")
                        nc.gpsimd.partition_broadcast(denb, den1, channels=64)
                        rec = rp.tile([64, 512], f32, tag="rec")
                        nc.vector.reciprocal_approx_fast(rec, denb)
                        r0 = sub * 64
                        nc.vector.tensor_mul(
                            recv[qc][r0:r0 + 64, 0, 2 * pair, :],
                            avr[0:64, :], rec
                        )
                        nc.vector.tensor_mul(
                            recv[qc][r0:r0 + 64, 0, 2 * pair + 1, :],
                            avi[0:64, :], rec
                        )
                    # send this pair's two chunks to the 3 XOR peers
                    if _NO_SEND:
                        return
                    for d in (1, 2, 3):
                        rdests = [None] * 8
                        rdests[d] = (0, d)
                        nc.gpsimd.remote_dma_broadcast(
                            recv[qc][:, d, 2 * pair:2 * pair + 2, :],
                            recv[qc][:, 0, 2 * pair:2 * pair + 2, :],
                            remote_sem=rsem[qc],
                            local_sem=lsem[qc],
                            rdests=rdests,
                        )
                    nc.gpsimd.trigger_dma(count=None)

                scores_pair(0)
                actchain_pair(0)
                avnorm_pair(0)
                scores_pair(1)
                actchain_pair(1)
                avnorm_pair(1)

            def oproj_qc(qc):
                # all 4 slots x 4 chunks must have arrived (per-qc semaphore)
                if qc == 0:
                    load_o_weights()
                if not (_NO_WAIT or _NO_SEND):
                    # 2 pairs x 3 peers x 2 incs per qc
                    with tc.tile_critical():
                        # own sends must be out first (12 broadcasts x +16):
                        # breaks the cross-core wait-before-send cycle, and
                        # the scheduler sim models local sems so it cannot
                        # hoist this above the triggers
                        nc.gpsimd.wait_ge(lsem[qc], 96)
                        nc.gpsimd.wait_ge(rsem[qc], 12)
                        nc.gpsimd.tensor_copy(
                            token_sb[:, qc:qc + 1], recv[qc][0:1, 3, 3, 0:1]
                        )
                po = {}
                for ri in range(2):
                    for odt in range(2):
                        po[(ri, odt)] = ps.tile(
                            [128, 512], f32, tag="ps", name=f"po{ri}{odt}_{qc}"
                        )
                if not (_NO_WAIT or _NO_SEND):
                    # tiny K=1 matmuls: RAW on token orders PE after the wait,
                    # WAW on each po bank orders the real accumulation after
                    # these; values wiped by the real group's start=True
                    for ri in range(2):
                        for odt in range(2):
                            nc.tensor.matmul(
                                po[(ri, odt)][0:1, 0:1],
                                lhsT=token_sb[:, qc:qc + 1],
                                rhs=token_sb[:, qc:qc + 1],
                                start=True, stop=True,
                                skip_group_check=True,
                            )
                for s in range(4):
                    for ch in range(4):
                        pair, ri = ch // 2, ch % 2
                        w = wo_sb["wor" if ri == 0 else "woi"]
                        for odt in range(2):
                            mm(po[(ri, odt)],
                               w[:, s, pair, odt * 128:(odt + 1) * 128],
                               recv[qc][:, s, ch, :],
                               start=(s == 0 and pair == 0),
                               stop=(s == 3 and pair == 1))
                for ri, odst in ((0, o_r), (1, o_i)):
                    for odt in range(2):
                        oo = op.tile([128, 512], f32, tag="oo")
                        nc.scalar.activation(
                            oo, po[(ri, odt)], AF.Identity, bias=bo_sb[(ri, odt)]
                        )
                        nc.sync.dma_start(
                            odst[odt * 128:(odt + 1) * 128,
                                 qc * 512:(qc + 1) * 512],
                            oo,
                        )

            token_sb = consts.tile([1, 2], bf16, tag="tok")

            load_x(0)
            proj_c(0)
            load_x(1)
            emit_barrier()
            attn_qc(0)
            proj_c(1)
            attn_qc(1)
            oproj_qc(0)
            oproj_qc(1)

    return nc


def _host_prep(inputs):
    """Fold ent/scale/bv on host; build per-core input maps in device layouts."""
    import ml_dtypes

    bf16 = ml_dtypes.bfloat16
    real = np.asarray(inputs["real"], np.float32)
    imag = np.asarray(inputs["imag"], np.float32)
    ent = np.asarray(inputs["ent"], np.float64)
    scale = 1.0 / math.sqrt(HD)

    def fold_w(W, do_ent, sc=1.0):
        W = np.asarray(W, np.float64).reshape(DIM, HEADS, HD)
        if do_ent:
            W = np.einsum("chd,hx->cxd", W, ent)
        return W * sc  # [DIM, HEADS, HD]

    def fold_b(b, do_ent, sc=1.0):
        b = np.asarray(b, np.float64).reshape(HEADS, HD)
        if do_ent:
            b = np.einsum("hd,hx->xd", b, ent)
        return b * sc

    Wq_r = fold_w(inputs["Wq_r"], True, scale)
    Wq_i = fold_w(inputs["Wq_i"], True, scale)
    Wk_r = fold_w(inputs["Wk_r"], True)
    Wk_i = fold_w(inputs["Wk_i"], True)
    Wv_r = fold_w(inputs["Wv_r"], False)
    Wv_i = fold_w(inputs["Wv_i"], False)
    bq_r = fold_b(inputs["bq_r"], True, scale)
    bq_i = fold_b(inputs["bq_i"], True, scale)
    bk_r = fold_b(inputs["bk_r"], True)
    bk_i = fold_b(inputs["bk_i"], True)
    Wo_r = np.asarray(inputs["Wo_r"], np.float64)
    Wo_i = np.asarray(inputs["Wo_i"], np.float64)
    bo_r = np.asarray(inputs["bo_r"], np.float64) + np.asarray(
        inputs["bv_r"], np.float64
    ) @ Wo_r
    bo_i = np.asarray(inputs["bo_i"], np.float64) + np.asarray(
        inputs["bv_i"], np.float64
    ) @ Wo_i

    strength = float(np.asarray(inputs["strength"]).reshape(-1)[0])
    temp = float(np.asarray(inputs["temp"]).reshape(-1)[0])
    gt = (1.0 / (1.0 + math.exp(-strength))) / max(temp, 0.01)

    # rope tables in pair-tile layout: row r (r%64 = d within head's 64 dims)
    rot_freqs = np.asarray(inputs["rot_freqs"], np.float64)  # [16]
    pos = np.arange(S, dtype=np.float64)
    emb = pos[:, None] * rot_freqs[None, :]  # [S, 16]
    cos_t = np.cos(emb)
    sin_t = np.sin(emb)
    cosd = np.ones((128, S), np.float64)
    sind = np.zeros((128, S), np.float64)
    for half in range(2):
        for d in range(ROTD):
            r = half * 64 + d
            cosd[r] = cos_t[:, d // 2]
            sind[r] = (-sin_t if d % 2 == 0 else sin_t)[:, d // 2]

    def pack_qk(Wf, g):
        # -> [128, 2, 8, 128]: [part, pair, kt, col]; col = (j//64)'th head of
        # pair, dim j%64 (dims 0..31 rot, 32..63 nr in natural order)
        Wc = Wf[:, 4 * g:4 * g + 4, :]  # [DIM, 4, 64]
        arr = Wc.reshape(8, 128, 2, 2, 64)  # [kt, part, pair, sub, d]
        arr = arr.transpose(1, 2, 0, 3, 4).reshape(128, 2, 8, 128)
        return np.ascontiguousarray(arr).astype(bf16)

    def pack_bqk_col(bf, g):
        # -> [128] rows: [h_even 64 dims | h_odd 64], per pair
        bc = bf[4 * g:4 * g + 4, :]  # [4, 64]
        return bc.reshape(2, 128)  # [pair, 128]

    in_maps = []
    for core in range(NCORES):
        b, g = core // 4, core % 4
        hs = slice(4 * g, 4 * g + 4)

        xT_r = real[b].T.astype(np.float64)  # [DIM, S]
        xT_i = imag[b].T.astype(np.float64)
        # -> [c, part, kt, tok]: partition-major so each SBUF row is one
        # contiguous 8KB DRAM run (1 DMA descriptor per row)
        xr = xT_r.reshape(8, 128, 2, 512).transpose(2, 1, 0, 3)
        xi = xT_i.reshape(8, 128, 2, 512).transpose(2, 1, 0, 3)

        bqk = np.zeros((128, 2, 4), np.float32)
        for kind, bf in enumerate((bq_r, bq_i, bk_r, bk_i)):
            pc = pack_bqk_col(bf, g)  # [pair, 128]
            bqk[:, :, kind] = pc.T

        wv_pack = {}
        for nm, Wf in (("wvr", Wv_r), ("wvi", Wv_i)):
            Wc = Wf[:, hs, :].reshape(DIM, 256)  # [DIM, 4*64]
            arr = Wc.reshape(8, 128, 256)
            wv_pack[nm] = np.ascontiguousarray(arr.transpose(1, 0, 2)).astype(bf16)

        wo_pack = {}
        for nm, Wf in (("wor", Wo_r), ("woi", Wo_i)):
            arr = np.zeros((128, 4, 2, 256), np.float64)
            for s_ in range(4):
                gp = g ^ s_
                for pair in range(2):
                    for sub in range(2):
                        h = 4 * gp + 2 * pair + sub
                        arr[sub * 64:(sub + 1) * 64, s_, pair, :] = Wf[
                            h * 64:(h + 1) * 64, g * ODC:(g + 1) * ODC
                        ]
            wo_pack[nm] = np.ascontiguousarray(arr).astype(bf16)

        bo = np.zeros((128, 2, 2), np.float32)
        for ri, bv in enumerate((bo_r, bo_i)):
            for odt in range(2):
                bo[:, ri, odt] = bv[g * ODC + odt * 128: g * ODC + (odt + 1) * 128]

        m = {
            "xr": xr.astype(bf16),
            "xi": xi.astype(bf16),
            "wqr": pack_qk(Wq_r, g),
            "wqi": pack_qk(Wq_i, g),
            "wkr": pack_qk(Wk_r, g),
            "wki": pack_qk(Wk_i, g),
            "wvr": wv_pack["wvr"],
            "wvi": wv_pack["wvi"],
            "wor": wo_pack["wor"],
            "woi": wo_pack["woi"],
            "bqk": bqk,
            "bo": bo,
            "cosd": cosd.astype(bf16),
            "sind": sind.astype(bf16),
        }
        in_maps.append(m)
    return in_maps, gt


def kernel(**inputs):
    from concourse import bass_utils

    in_maps, gt = _host_prep(inputs)
    nc = _build(gt)
    nc.finalize()
    res = bass_utils.run_bass_kernel_spmd(nc, in_maps, core_ids=list(range(NCORES)))
    out_r = np.empty((B, S, DIM), np.float32)
    out_i = np.empty((B, S, DIM), np.float32)
    for core in range(NCORES):
        b, g = core // 4, core % 4
        out_r[b, :, g * ODC:(g + 1) * ODC] = res.results[core]["o_r"].T
        out_i[b, :, g * ODC:(g + 1) * ODC] = res.results[core]["o_i"].T
    return np.stack([out_r, out_i], axis=0)


# revision 23
# speedup vs baseline: 1.0637x; 1.0637x over previous
"""Trainium2 Bass kernel for the EntangledInterferenceLayer problem (v2).

Math transformations on host (numpy float64, exact up to fp rounding):
  * HxH entanglement mix commutes with RoPE -> folded into Q/K weights+biases.
  * Per-head phase shift cancels in q*conj(k) -> dropped.
  * 1/sqrt(head_dim) folded into Q weights/bias.
  * V-projection bias contributes bv @ Wo to every row -> folded into out bias.

Sharding (8 cores): core = (batch b, head-group g of 4 heads). Device compute
in bf16 (fp32 PSUM/softmax internals); rel tolerance is 2e-2.

v2 design vs the original baseline:
  * Projections computed per head-PAIR: stationary [128, h0|h1 x 64 dims],
    PSUM rows = [h0 rot|nr | h1 rot|nr]; bias added by ScalarE (Identity with
    per-partition bias AP); RoPE applied by DVE (stream_shuffle pairswap +
    cos/sin tables with 1/0 rows for non-rot dims) writing directly into the
    per-head Q/K1 stacks via partition-offset-shifted adds. No bias matmuls,
    no SBUF->SBUF staging DMAs.
  * K2 = [-ki | kr] built with two 64-row DVE copies per head.
  * V written straight from PSUM into the persistent V tile (strided ACT copy).
  * All weights/x pre-packed on host into the exact SBUF layouts (contiguous
    DMA), loaded once.
  * Attention trimmed to the causal support: kv-tile (q >= 128*off) column
    ranges only (~25% less score/softmax/AV work); sqrt/exp batched per
    head-pair so ACT reloads tables only 2x per pair.
  * Softmax denominator: ones-column in Vr; reciprocal via gpsimd
    partition_broadcast + reciprocal_approx_fast; normalized outputs written
    into the local slot of the recv buffer as pure-real/imag 2-head chunks.
  * Comms: remote_dma_broadcast (XOR-relative dests, slots 1-3) pushes each
    pair's normalized output [128, 2, 512] bf16 directly into the 3 peers'
    recv SBUF slots; arrival via per-qc remote semaphores (+2/send). The
    receive side waits inside a tile_critical (the Tile scheduler's
    single-core sim cannot model remote arrivals): first on the local send
    semaphore (orders own sends before the wait - breaks the cross-core
    wait-before-send deadlock the scheduler cannot see), then on the remote
    arrival count; a token tile written after the wait plus tiny K=1 dummy
    matmuls into each output PSUM bank (wiped by the real group's
    start=True) give the tensor engine a pure data-flow ordering, since a
    standalone PE wait instruction inside a critical section hangs the
    device. A tiny AllReduce barrier + sem_clear at kernel start makes
    re-execution of the NEFF safe; sends are ordered after the barrier via
    data edges (barrier result DMA'd into each payload chunk corner,
    overwritten by the normalize muls).
  * Output projection contracts recv chunks (K=128 fully used since chunks
    are pure real/imag) with per-core slot-permuted Wo (host knows each
    core's XOR peer map), + ScalarE bias, f32 out.
"""

import math
import os

import numpy as np

_NO_SEND = os.environ.get("ANT_NO_SEND") == "1"
_NO_WAIT = os.environ.get("ANT_NO_WAIT") == "1"

B, S, DIM = 2, 1024, 1024
HEADS, HD, ROTD = 16, 64, 32
GH = 4  # heads per core
ODC = 256  # out-dim columns per core
NCORES = 8

_PAIRSWAP = [i ^ 1 for i in range(32)]


def _register_magsq():
    """Register a fused custom DVE op: out = (in0^2 + in1^2) * imm2."""
    import numpy as np
    from concourse import dve_ops as DO
    from concourse.dve_spec import Spec, Src0, Src1, C2, sq, lower

    if "ANT_MAGSQ" in DO._SUB_OPCODE_FOR_NAME:
        return next(o for o in DO.OPS if o.name == "ANT_MAGSQ")
    spec = Spec(
        body=(sq(Src0) + sq(Src1)) * C2,
        reference=lambda in0, in1, s0, s1, imm2: (
            in0.astype(np.float32) ** 2 + in1.astype(np.float32) ** 2
        )
        * np.float32(imm2),
    )
    opcode = DO._CUSTOM_DVE_ROW_BASE + len(DO.OPS)
    DO._SUB_OPCODE_FOR_NAME["ANT_MAGSQ"] = opcode
    shas = {}
    for ver in ("v3", "v4"):
        try:
            s = DO.DveOpSpec(
                name="ANT_MAGSQ", opcode=opcode, uops=lower(spec, ver=ver), rd1_en=True
            )
            shas[ver] = s.sha(ver)
        except Exception:
            pass
    op = DO.DveOp("ANT_MAGSQ", spec, subdim=False, uops_sha=shas)
    DO.OPS.append(op)
    DO.CUSTOM_DVE_SPECS["ANT_MAGSQ"] = spec
    return op


def _build(gt: float, groups=None):
    import concourse.mybir as mybir
    import concourse.tile as tile
    from concourse import bacc

    f32 = mybir.dt.float32
    bf16 = mybir.dt.bfloat16
    AF = mybir.ActivationFunctionType
    magsq = _register_magsq()

    nc = bacc.Bacc("TRN2", target_bir_lowering=False, num_devices=NCORES)
    if groups is None:
        groups = [[0, 1, 2, 3], [4, 5, 6, 7]]

    xr_d = nc.dram_tensor("xr", [2, 128, 8, 512], bf16, kind="ExternalInput")
    xi_d = nc.dram_tensor("xi", [2, 128, 8, 512], bf16, kind="ExternalInput")
    wqk_d = {
        nm: nc.dram_tensor(nm, [128, 2, 8, 128], bf16, kind="ExternalInput")
        for nm in ("wqr", "wqi", "wkr", "wki")
    }
    wv_d = {
        nm: nc.dram_tensor(nm, [128, 8, 256], bf16, kind="ExternalInput")
        for nm in ("wvr", "wvi")
    }
    wo_d = {
        nm: nc.dram_tensor(nm, [128, 4, 2, 256], bf16, kind="ExternalInput")
        for nm in ("wor", "woi")
    }
    bqk_d = nc.dram_tensor("bqk", [128, 2, 4], f32, kind="ExternalInput")
    bo_d = nc.dram_tensor("bo", [128, 2, 2], f32, kind="ExternalInput")
    cos_d = nc.dram_tensor("cosd", [128, 1024], bf16, kind="ExternalInput")
    sin_d = nc.dram_tensor("sind", [128, 1024], bf16, kind="ExternalInput")
    o_r = nc.dram_tensor("o_r", [ODC, S], f32, kind="ExternalOutput")
    o_i = nc.dram_tensor("o_i", [ODC, S], f32, kind="ExternalOutput")

    rsem = [nc.alloc_semaphore("rsem0"), nc.alloc_semaphore("rsem1")]
    lsem = [nc.alloc_semaphore("lsem0"), nc.alloc_semaphore("lsem1")]

    def mm(out, lhsT, rhs, start, stop):
        nc.tensor.matmul(out, lhsT=lhsT, rhs=rhs, start=start, stop=stop)

    with tile.TileContext(nc) as tc:
        with (
            tc.tile_pool(name="consts", bufs=1) as consts,
            tc.tile_pool(name="persist", bufs=1) as persist,
            tc.tile_pool(name="xp", bufs=4) as xp,
            tc.tile_pool(name="ptmp", bufs=3) as ptmp,
            tc.tile_pool(name="cp", bufs=4) as cp,
            tc.tile_pool(name="sqp", bufs=17) as sqp,
            tc.tile_pool(name="etp", bufs=17) as etp,
            tc.tile_pool(name="rp", bufs=2) as rp,
            tc.tile_pool(name="op", bufs=2) as op,
            tc.tile_pool(name="ps", bufs=8, space="PSUM") as ps,
            tc.tile_pool(name="dram", bufs=1, space="DRAM") as dram,
        ):
            # ---- persistent SBUF state ----
            Q = persist.tile([128, GH, S], bf16, tag="Q")
            K1 = persist.tile([128, GH, S], bf16, tag="K1")
            K2 = persist.tile([128, GH, S], bf16, tag="K2")
            Vr = persist.tile([128, 8, GH, 65], bf16, tag="Vr")
            Vi = persist.tile([128, 8, GH, 64], bf16, tag="Vi")
            recv = [
                persist.tile([128, 4, 4, 512], bf16, tag=f"recv{qc}",
                             name=f"recv{qc}")
                for qc in range(2)
            ]

            # ---- init: sem clear + cross-core barrier (re-exec safety) ----
            nc.gpsimd.sem_clear(rsem[0])
            nc.gpsimd.sem_clear(rsem[1])
            nc.gpsimd.sem_clear(lsem[0])
            nc.gpsimd.sem_clear(lsem[1])
            def emit_barrier():
                bar_in = dram.tile([1, 4], f32, tag="bar_in", name="bar_in")
                bar_out = dram.tile([1, 4], f32, tag="bar_out", name="bar_out")
                barsb = consts.tile([1, 4], f32, tag="barsb")
                nc.vector.memset(barsb, 1.0)
                nc.sync.dma_start(bar_in, barsb)
                nc.gpsimd.collective_compute(
                    "AllReduce",
                    mybir.AluOpType.add,
                    replica_groups=groups,
                    ins=[bar_in[:].opt()],
                    outs=[bar_out[:].opt()],
                )
                # data edges: gate every send payload chunk on the barrier
                for qc in range(2):
                    for ch in range(4):
                        nc.sync.dma_start(
                            recv[qc][0:1, 0, ch, 0:8].bitcast(f32), bar_out
                        )

            # ---- constants / weights (loaded once) ----
            nc.vector.memset(Vr[:, :, :, 64:65], 1.0)
            eps_t = consts.tile([128, 1], f32, tag="eps")
            nc.vector.memset(eps_t, 1e-6 * float(gt) * float(gt))
            cos_sb = consts.tile([128, 1024], bf16, tag="cos")
            nc.sync.dma_start(cos_sb, cos_d[:, :])
            sin_sb = consts.tile([128, 1024], bf16, tag="sin")
            nc.sync.dma_start(sin_sb, sin_d[:, :])
            bqk_sb = {}
            for pair in range(2):
                for kind in range(4):
                    t = consts.tile([128, 1], f32, tag=f"bqk{pair}{kind}",
                                    name=f"bqk{pair}{kind}")
                    nc.sync.dma_start(t, bqk_d[:, pair:pair + 1, kind])
                    bqk_sb[(pair, kind)] = t
            bo_sb = {}
            for ri in range(2):
                for odt in range(2):
                    t = consts.tile([128, 1], f32, tag=f"bo{ri}{odt}",
                                    name=f"bo{ri}{odt}")
                    nc.sync.dma_start(t, bo_d[:, ri:ri + 1, odt])
                    bo_sb[(ri, odt)] = t
            wqk_sb = {}
            wv_sb = {}
            wo_sb = {}

            def load_qk_weights():
                for nm in wqk_d:
                    t = consts.tile([128, 2, 8, 128], bf16, tag=nm, name=nm)
                    nc.sync.dma_start(t, wqk_d[nm][:, :, :, :])
                    wqk_sb[nm] = t

            def load_v_weights():
                for nm in wv_d:
                    t = consts.tile([128, 8, 256], bf16, tag=nm, name=nm)
                    nc.sync.dma_start(t, wv_d[nm][:, :, :])
                    wv_sb[nm] = t

            def load_o_weights():
                for nm in wo_d:
                    t = consts.tile([128, 4, 2, 256], bf16, tag=nm, name=nm)
                    nc.sync.dma_start(t, wo_d[nm][:, :, :, :])
                    wo_sb[nm] = t

            def proj_pair(pair, w_sb, bias_kind, x_tiles, dst, half, csl):
                """One [128,512] PSUM pair-tile -> rope -> dst rows."""
                pst = ps.tile([128, 512], f32, tag="ps")
                for kt in range(8):
                    mm(pst, w_sb[:, pair, kt, :], x_tiles[kt],
                       start=(kt == 0), stop=(kt == 7))
                tb = ptmp.tile([128, 512], bf16, tag="tb")
                nc.scalar.activation(
                    tb, pst, AF.Identity, bias=bqk_sb[(pair, bias_kind)]
                )
                sh = ptmp.tile([128, 512], bf16, tag="sh")
                nc.vector.stream_shuffle(sh, tb, mask=_PAIRSWAP)
                nc.vector.tensor_mul(sh, sh, sin_sb[:, csl])
                t2 = ptmp.tile([128, 512], bf16, tag="t2")
                nc.vector.tensor_mul(t2, tb, cos_sb[:, csl])
                h0, h1 = 2 * pair, 2 * pair + 1
                r0 = half * 64
                nc.vector.tensor_add(
                    dst[r0:r0 + 64, h0, csl], t2[0:64, :], sh[0:64, :]
                )
                nc.vector.tensor_add(
                    dst[r0:r0 + 64, h1, csl], t2[64:128, :], sh[64:128, :]
                )

            x_tiles = {}

            def load_x(c):
                xr_b = xp.tile([128, 8, 512], bf16, tag="xt", name=f"xr{c}")
                xi_b = xp.tile([128, 8, 512], bf16, tag="xt", name=f"xi{c}")
                # split loads so the first matmul only waits for its k-chunks
                for q4 in range(4):
                    nc.sync.dma_start(
                        xr_b[:, 2 * q4:2 * q4 + 2, :],
                        xr_d[c, :, 2 * q4:2 * q4 + 2, :],
                    )
                for q4 in range(4):
                    nc.sync.dma_start(
                        xi_b[:, 2 * q4:2 * q4 + 2, :],
                        xi_d[c, :, 2 * q4:2 * q4 + 2, :],
                    )
                x_tiles[c] = (xr_b, xi_b)

            def proj_c(c):
                csl = slice(c * 512, (c + 1) * 512)
                xr_b, xi_b = x_tiles[c]
                xr_t = [xr_b[:, kt, :] for kt in range(8)]
                xi_t = [xi_b[:, kt, :] for kt in range(8)]
                if c == 0:
                    load_qk_weights()
                for pair in range(2):
                    proj_pair(pair, wqk_sb["wqr"], 0, xr_t, Q, 0, csl)
                    proj_pair(pair, wqk_sb["wqi"], 1, xi_t, Q, 1, csl)
                    proj_pair(pair, wqk_sb["wkr"], 2, xr_t, K1, 0, csl)
                    proj_pair(pair, wqk_sb["wki"], 3, xi_t, K1, 1, csl)
                    for h in (2 * pair, 2 * pair + 1):
                        nc.vector.tensor_scalar_mul(
                            K2[0:64, h, csl], K1[64:128, h, csl], -1.0
                        )
                        nc.vector.tensor_copy(K2[64:128, h, csl], K1[0:64, h, csl])
                if c == 0:
                    load_v_weights()
                # V projections: out [tok, vdim] (x as stationary)
                for w_sb, Vt, x_t, w65 in (
                    (wv_sb["wvr"], Vr, xr_t, True),
                    (wv_sb["wvi"], Vi, xi_t, False),
                ):
                    for tl in range(4):
                        tt = c * 4 + tl
                        pv = ps.tile([128, 256], f32, tag="ps")
                        for kt in range(8):
                            mm(pv, x_t[kt][:, tl * 128:(tl + 1) * 128],
                               w_sb[:, kt, :], start=(kt == 0), stop=(kt == 7))
                        dst = Vt[:, tt, :, 0:64] if w65 else Vt[:, tt, :, :]
                        nc.scalar.activation(dst, pv, AF.Identity)

            def qrange(qc, kvt):
                off = kvt - qc * 4
                qlo = 128 * off if off > 0 else 0
                return off, qlo

            def attn_qc(qc):
                nkv = 4 * (qc + 1)
                qbase = qc * 512
                tiles = {0: [], 1: []}
                ets = {}

                def scores_pair(pair):
                    for h in (2 * pair, 2 * pair + 1):
                        for kvt in range(nkv):
                            off, qlo = qrange(qc, kvt)
                            qs = slice(qbase + qlo, qbase + 512)
                            ksl = slice(kvt * 128, (kvt + 1) * 128)
                            psr = ps.tile([128, 512], f32, tag="ps")
                            mm(psr[:, qlo:], K1[:, h, ksl], Q[:, h, qs],
                               start=True, stop=True)
                            psi = ps.tile([128, 512], f32, tag="ps")
                            mm(psi[:, qlo:], K2[:, h, ksl], Q[:, h, qs],
                               start=True, stop=True)
                            c1 = cp.tile([128, 512], bf16, tag="c1")
                            nc.vector.tensor_copy(c1[:, qlo:], psi[:, qlo:])
                            sq = sqp.tile([128, 512], bf16, tag="sq")
                            nc.vector._custom_dve(
                                magsq, out=sq[:, qlo:], in0=psr[:, qlo:],
                                in1=c1[:, qlo:], imm2=float(gt) * float(gt),
                            )
                            tiles[pair].append((h, kvt, qlo, sq))

                def actchain_pair(pair):
                    # batched ACT passes (one table load per function)
                    for h, kvt, qlo, sq in tiles[pair]:
                        nc.scalar.activation(
                            sq[:, qlo:], sq[:, qlo:], AF.Sqrt, bias=eps_t
                        )
                    for h, kvt, qlo, sq in tiles[pair]:
                        et = etp.tile([128, 512], bf16, tag="et")
                        nc.scalar.activation(et[:, qlo:], sq[:, qlo:], AF.Exp)
                        ets[(h, kvt)] = et

                def avnorm_pair(pair):
                    for h, kvt, qlo, sq in tiles[pair]:
                        if kvt - qc * 4 >= 0:
                            et = ets[(h, kvt)]
                            nc.gpsimd.affine_select(
                                out=et[:, qlo:],
                                in_=et[:, qlo:],
                                compare_op=mybir.AluOpType.is_ge,
                                fill=0.0,
                                base=0,
                                channel_multiplier=-1,
                                pattern=[[1, 512 - qlo]],
                            )
                    for sub, h in enumerate((2 * pair, 2 * pair + 1)):
                        avr = ps.tile([65, 512], f32, tag="ps")
                        avi = ps.tile([64, 512], f32, tag="ps")
                        for kvt in range(nkv):
                            off, qlo = qrange(qc, kvt)
                            et = ets[(h, kvt)]
                            mm(avr[:, qlo:], Vr[:, kvt, h, :], et[:, qlo:],
                               start=(kvt == 0), stop=(kvt == nkv - 1))
                            mm(avi[:, qlo:], Vi[:, kvt, h, :], et[:, qlo:],
                               start=(kvt == 0), stop=(kvt == nkv - 1))
                        den1 = rp.tile([1, 512], f32, tag="den1")
                        nc.vector.tensor_copy(den1, avr[64:65, :])
                        denb = rp.tile([64, 512], f32, tag="denb")
                        nc.gpsimd.partition_broadcast(denb, den1, channels=64)
                        rec = rp.tile([64, 512], f32, tag="rec")
                        nc.vector.reciprocal_approx_fast(rec, denb)
                        r0 = sub * 64
                        nc.vector.tensor_mul(
                            recv[qc][r0:r0 + 64, 0, 2 * pair, :],
                            avr[0:64, :], rec
                        )
                        nc.vector.tensor_mul(
                            recv[qc][r0:r0 + 64, 0, 2 * pair + 1, :],
                            avi[0:64, :], rec
                        )
                    # send this pair's two chunks to the 3 XOR peers
                    if _NO_SEND:
                        return
                    for d in (1, 2, 3):
                        rdests = [None] * 8
                        rdests[d] = (0, d)
                        nc.gpsimd.remote_dma_broadcast(
                            recv[qc][:, d, 2 * pair:2 * pair + 2, :],
                            recv[qc][:, 0, 2 * pair:2 * pair + 2, :],
                            remote_sem=rsem[qc],
                            local_sem=lsem[qc],
                            rdests=rdests,
                        )
                    nc.gpsimd.trigger_dma(count=None)

                scores_pair(0)
                actchain_pair(0)
                avnorm_pair(0)
                scores_pair(1)
                actchain_pair(1)
                avnorm_pair(1)

            def oproj_qc(qc):
                # all 4 slots x 4 chunks must have arrived (per-qc semaphore)
                if qc == 0:
                    load_o_weights()
                if not (_NO_WAIT or _NO_SEND):
                    # 2 pairs x 3 peers x 2 incs per qc
                    with tc.tile_critical():
                        # own sends must be out first (12 broadcasts x +16):
                        # breaks the cross-core wait-before-send cycle, and
                        # the scheduler sim models local sems so it cannot
                        # hoist this above the triggers
                        nc.gpsimd.wait_ge(lsem[qc], 96)
                        nc.gpsimd.wait_ge(rsem[qc], 12)
                        nc.gpsimd.tensor_copy(
                            token_sb[:, qc:qc + 1], recv[qc][0:1, 3, 3, 0:1]
                        )
                po = {}
                for ri in range(2):
                    for odt in range(2):
                        po[(ri, odt)] = ps.tile(
                            [128, 512], f32, tag="ps", name=f"po{ri}{odt}_{qc}"
                        )
                if not (_NO_WAIT or _NO_SEND):
                    # tiny K=1 matmuls: RAW on token orders PE after the wait,
                    # WAW on each po bank orders the real accumulation after
                    # these; values wiped by the real group's start=True
                    for ri in range(2):
                        for odt in range(2):
                            nc.tensor.matmul(
                                po[(ri, odt)][0:1, 0:1],
                                lhsT=token_sb[:, qc:qc + 1],
                                rhs=token_sb[:, qc:qc + 1],
                                start=True, stop=True,
                                skip_group_check=True,
                            )
                for s in range(4):
                    for ch in range(4):
                        pair, ri = ch // 2, ch % 2
                        w = wo_sb["wor" if ri == 0 else "woi"]
                        for odt in range(2):
                            mm(po[(ri, odt)],
                               w[:, s, pair, odt * 128:(odt + 1) * 128],
                               recv[qc][:, s, ch, :],
                               start=(s == 0 and pair == 0),
                               stop=(s == 3 and pair == 1))
                for ri, odst in ((0, o_r), (1, o_i)):
                    for odt in range(2):
                        oo = op.tile([128, 512], f32, tag="oo")
                        nc.scalar.activation(
                            oo, po[(ri, odt)], AF.Identity, bias=bo_sb[(ri, odt)]
                        )
                        nc.sync.dma_start(
                            odst[odt * 128:(odt + 1) * 128,
                                 qc * 512:(qc + 1) * 512],
                            oo,
                        )

            token_sb = consts.tile([1, 2], bf16, tag="tok")

            load_x(0)
            proj_c(0)
            load_x(1)
            emit_barrier()
            attn_qc(0)
            proj_c(1)
            attn_qc(1)
            oproj_qc(0)
            oproj_qc(1)

    return nc


def _host_prep(inputs):
    """Fold ent/scale/bv on host; build per-core input maps in device layouts."""
    import ml_dtypes

    bf16 = ml_dtypes.bfloat16
    real = np.asarray(inputs["real"], np.float32)
    imag = np.asarray(inputs["imag"], np.float32)
    ent = np.asarray(inputs["ent"], np.float64)
    scale = 1.0 / math.sqrt(HD)

    def fold_w(W, do_ent, sc=1.0):
        W = np.asarray(W, np.float64).reshape(DIM, HEADS, HD)
        if do_ent:
            W = np.einsum("chd,hx->cxd", W, ent)
        return W * sc  # [DIM, HEADS, HD]

    def fold_b(b, do_ent, sc=1.0):
        b = np.asarray(b, np.float64).reshape(HEADS, HD)
        if do_ent:
            b = np.einsum("hd,hx->xd", b, ent)
        return b * sc

    Wq_r = fold_w(inputs["Wq_r"], True, scale)
    Wq_i = fold_w(inputs["Wq_i"], True, scale)
    Wk_r = fold_w(inputs["Wk_r"], True)
    Wk_i = fold_w(inputs["Wk_i"], True)
    Wv_r = fold_w(inputs["Wv_r"], False)
    Wv_i = fold_w(inputs["Wv_i"], False)
    bq_r = fold_b(inputs["bq_r"], True, scale)
    bq_i = fold_b(inputs["bq_i"], True, scale)
    bk_r = fold_b(inputs["bk_r"], True)
    bk_i = fold_b(inputs["bk_i"], True)
    Wo_r = np.asarray(inputs["Wo_r"], np.float64)
    Wo_i = np.asarray(inputs["Wo_i"], np.float64)
    bo_r = np.asarray(inputs["bo_r"], np.float64) + np.asarray(
        inputs["bv_r"], np.float64
    ) @ Wo_r
    bo_i = np.asarray(inputs["bo_i"], np.float64) + np.asarray(
        inputs["bv_i"], np.float64
    ) @ Wo_i

    strength = float(np.asarray(inputs["strength"]).reshape(-1)[0])
    temp = float(np.asarray(inputs["temp"]).reshape(-1)[0])
    gt = (1.0 / (1.0 + math.exp(-strength))) / max(temp, 0.01)

    # rope tables in pair-tile layout: row r (r%64 = d within head's 64 dims)
    rot_freqs = np.asarray(inputs["rot_freqs"], np.float64)  # [16]
    pos = np.arange(S, dtype=np.float64)
    emb = pos[:, None] * rot_freqs[None, :]  # [S, 16]
    cos_t = np.cos(emb)
    sin_t = np.sin(emb)
    cosd = np.ones((128, S), np.float64)
    sind = np.zeros((128, S), np.float64)
    for half in range(2):
        for d in range(ROTD):
            r = half * 64 + d
            cosd[r] = cos_t[:, d // 2]
            sind[r] = (-sin_t if d % 2 == 0 else sin_t)[:, d // 2]

    def pack_qk(Wf, g):
        # -> [128, 2, 8, 128]: [part, pair, kt, col]; col = (j//64)'th head of
        # pair, dim j%64 (dims 0..31 rot, 32..63 nr in natural order)
        Wc = Wf[:, 4 * g:4 * g + 4, :]  # [DIM, 4, 64]
        arr = Wc.reshape(8, 128, 2, 2, 64)  # [kt, part, pair, sub, d]
        arr = arr.transpose(1, 2, 0, 3, 4).reshape(128, 2, 8, 128)
        return np.ascontiguousarray(arr).astype(bf16)

    def pack_bqk_col(bf, g):
        # -> [128] rows: [h_even 64 dims | h_odd 64], per pair
        bc = bf[4 * g:4 * g + 4, :]  # [4, 64]
        return bc.reshape(2, 128)  # [pair, 128]

    in_maps = []
    for core in range(NCORES):
        b, g = core // 4, core % 4
        hs = slice(4 * g, 4 * g + 4)

        xT_r = real[b].T.astype(np.float64)  # [DIM, S]
        xT_i = imag[b].T.astype(np.float64)
        # -> [c, part, kt, tok]: partition-major so each SBUF row is one
        # contiguous 8KB DRAM run (1 DMA descriptor per row)
        xr = xT_r.reshape(8, 128, 2, 512).transpose(2, 1, 0, 3)
        xi = xT_i.reshape(8, 128, 2, 512).transpose(2, 1, 0, 3)

        bqk = np.zeros((128, 2, 4), np.float32)
        for kind, bf in enumerate((bq_r, bq_i, bk_r, bk_i)):
            pc = pack_bqk_col(bf, g)  # [pair, 128]
            bqk[:, :, kind] = pc.T

        wv_pack = {}
        for nm, Wf in (("wvr", Wv_r), ("wvi", Wv_i)):
            Wc = Wf[:, hs, :].reshape(DIM, 256)  # [DIM, 4*64]
            arr = Wc.reshape(8, 128, 256)
            wv_pack[nm] = np.ascontiguousarray(arr.transpose(1, 0, 2)).astype(bf16)

        wo_pack = {}
        for nm, Wf in (("wor", Wo_r), ("woi", Wo_i)):
            arr = np.zeros((128, 4, 2, 256), np.float64)
            for s_ in range(4):
                gp = g ^ s_
                for pair in range(2):
                    for sub in range(2):
                        h = 4 * gp + 2 * pair + sub
                        arr[sub * 64:(sub + 1) * 64, s_, pair, :] = Wf[
                            h * 64:(h + 1) * 64, g * ODC:(g + 1) * ODC
                        ]
            wo_pack[nm] = np.ascontiguousarray(arr).astype(bf16)

        bo = np.zeros((128, 2, 2), np.float32)
        for ri, bv in enumerate((bo_r, bo_i)):
            for odt in range(2):
                bo[:, ri, odt] = bv[g * ODC + odt * 128: g * ODC + (odt + 1) * 128]

        m = {
            "xr": xr.astype(bf16),
            "xi": xi.astype(bf16),
            "wqr": pack_qk(Wq_r, g),
            "wqi": pack_qk(Wq_i, g),
            "wkr": pack_qk(Wk_r, g),
            "wki": pack_qk(Wk_i, g),
            "wvr": wv_pack["wvr"],
            "wvi": wv_pack["wvi"],
            "wor": wo_pack["wor"],
            "woi": wo_pack["woi"],
            "bqk": bqk,
            "bo": bo,
            "cosd": cosd.astype(bf16),
            "sind": sind.astype(bf16),
        }
        in_maps.append(m)
    return in_maps, gt


def kernel(**inputs):
    from concourse import bass_utils

    in_maps, gt = _host_prep(inputs)
    nc = _build(gt)
    nc.finalize()
    res = bass_utils.run_bass_kernel_spmd(nc, in_maps, core_ids=list(range(NCORES)))
    out_r = np.empty((B, S, DIM), np.float32)
    out_i = np.empty((B, S, DIM), np.float32)
    for core in range(NCORES):
        b, g = core // 4, core % 4
        out_r[b, :, g * ODC:(g + 1) * ODC] = res.results[core]["o_r"].T
        out_i[b, :, g * ODC:(g + 1) * ODC] = res.results[core]["o_i"].T
    return np.stack([out_r, out_i], axis=0)
